# revision 1
# baseline (speedup 1.0000x reference)
"""BinaryTreeLSTM on 8 Trainium2 NeuronCores.

Data-parallel over the leaf batch: core d owns leaves [1024d, 1024d+1024)
and folds its subtree through 10 merge levels; the 8 per-core roots are
AllGathered and the final 3 levels run replicated on every core.

Two matmul regimes (fp32r operands, single-pass PE):
- Feature-major (leaf, B=512, B=256 levels): weights stationary, nodes
  on the moving free dim. State h is kept as [128, 2 chunks * B] with
  even/odd children split into separate tiles so weight loads and reads
  stay contiguous.
- Node-major (B <= 128 levels): h chunks stationary (tiny weight loads),
  W streams as the moving operand in 512-wide chunks. Gates/c/h are
  node-major [B, 256]; h is transposed back to feature-major via PE
  transposes for the next level, and lc/rc come from partition-strided
  SBUF DMAs of the previous node-major c.
"""

import numpy as np

IN_DIM = 300
MEM_DIM = 256
N_LEAVES = 8192
N_CORES = 8
LPC = N_LEAVES // N_CORES  # 1024 leaves per core

# FM-gate m-chunk (5-gate [u,i,lf,rf,o] x 2 halves) -> column of the
# [128, 8] feature-major pad_xg ([cx,ix,fx,ox]; lf and rf share fx)
_PXCOL = [0, 1, 2, 3, 4, 5, 4, 5, 6, 7]
# node-major 5-gate px layout offsets into the 4-gate [1,1024] px row
_PX5SRC = [0, 256, 512, 512, 768]

_CACHE = {}


def _build():
    import concourse.bacc as bacc
    import concourse.mybir as mybir
    import concourse.tile as tile

    f32 = mybir.dt.float32
    f32r = mybir.dt.float32r
    AF = mybir.ActivationFunctionType

    nc = bacc.Bacc("TRN2", target_bir_lowering=False, debug=False,
                   num_devices=N_CORES)

    embsT = nc.dram_tensor("embsT", [IN_DIM, LPC], f32r, kind="ExternalInput").ap()
    WxT = nc.dram_tensor("WxT", [IN_DIM, 1024], f32r, kind="ExternalInput").ap()
    WlT = nc.dram_tensor("WlT", [MEM_DIM, 1280], f32r, kind="ExternalInput").ap()
    WrT = nc.dram_tensor("WrT", [MEM_DIM, 1280], f32r, kind="ExternalInput").ap()
    bxr = nc.dram_tensor("bxr", [1, 1024], f32, kind="ExternalInput").ap()
    padT = nc.dram_tensor("padT", [IN_DIM, 1], f32r, kind="ExternalInput").ap()
    eye_in = nc.dram_tensor("eye_in", [128, 128], f32, kind="ExternalInput").ap()
    ones_in = nc.dram_tensor("ones_in", [1, 128], f32r, kind="ExternalInput").ap()
    out = nc.dram_tensor("out", [2, MEM_DIM], f32, kind="ExternalOutput").ap()

    with tile.TileContext(nc) as tc:
        with (
            tc.tile_pool(name="const", bufs=1) as const,
            tc.tile_pool(name="state", bufs=2) as state,
            tc.tile_pool(name="gates", bufs=2) as gates,
            tc.tile_pool(name="psum", bufs=2, space="PSUM") as psum,
            tc.tile_pool(name="dram", bufs=1, space="DRAM") as dram,
        ):
            v2 = lambda t: t.rearrange("p (c n) -> p c n", c=2)

            # ---- constants ----
            WxT_sb = const.tile([128, 3 * 1024], f32r)
            embsT_sb = const.tile([128, 3 * LPC], f32r)
            for k in range(3):
                r = 128 if k < 2 else IN_DIM - 256
                nc.sync.dma_start(WxT_sb[0:r, k * 1024:(k + 1) * 1024],
                                  WxT[128 * k:128 * k + r, :])
                nc.sync.dma_start(embsT_sb[0:r, k * LPC:(k + 1) * LPC],
                                  embsT[128 * k:128 * k + r, :])
            WlT_sb = const.tile([128, 2 * 1280], f32r)
            WrT_sb = const.tile([128, 2 * 1280], f32r)
            for k in range(2):
                nc.sync.dma_start(WlT_sb[:, k * 1280:(k + 1) * 1280],
                                  WlT[128 * k:128 * (k + 1), :])
                nc.sync.dma_start(WrT_sb[:, k * 1280:(k + 1) * 1280],
                                  WrT[128 * k:128 * (k + 1), :])
            bx_sb = const.tile([1, 1024], f32)
            nc.sync.dma_start(bx_sb[:, :], bxr[:, :])
            bx_fm = const.tile([128, 8], f32)
            nc.sync.dma_start(bx_fm[:, :],
                              bxr.rearrange("o (m p) -> p (o m)", p=128))
            padT_sb = const.tile([128, 3], f32r)
            for k in range(3):
                r = 128 if k < 2 else IN_DIM - 256
                nc.sync.dma_start(padT_sb[0:r, k:k + 1], padT[128 * k:128 * k + r, :])
            eye_sb = const.tile([128, 128], f32)
            nc.sync.dma_start(eye_sb[:, :], eye_in[:, :])
            ones_sb = const.tile([1, 128], f32r)
            nc.sync.dma_start(ones_sb[:, :], ones_in[:, :])

            # ---- leaf phase ----
            c0 = state.tile([128, 2 * LPC], f32, tag="c")
            hev = state.tile([128, 2 * 512], f32r, tag="hev", name="hev_leaf")
            hod = state.tile([128, 2 * 512], f32r, tag="hod", name="hod_leaf")
            c0_3, hev3, hod3 = v2(c0), v2(hev), v2(hod)
            GL = 512
            for sg in range(LPC // GL):
                xg = {}
                for gname, gm in (("u", 0), ("i", 1), ("o", 3)):
                    t = psum.tile([128, 2 * GL], f32, tag="g", name=f"x{gname}{sg}")
                    for half in range(2):
                        m = gm * 2 + half
                        dst = t[:, half * GL:(half + 1) * GL]
                        for ki in range(3):
                            r = 128 if ki < 2 else IN_DIM - 256
                            nc.tensor.matmul(
                                dst,
                                WxT_sb[0:r, ki * 1024 + m * 128:
                                       ki * 1024 + (m + 1) * 128],
                                embsT_sb[0:r, ki * LPC + sg * GL:
                                         ki * LPC + (sg + 1) * GL],
                                start=(ki == 0), stop=(ki == 2))
                    xg[gname] = t
                ut = gates.tile([128, 2 * GL], f32, tag="u", name=f"u{sg}")
                it = gates.tile([128, 2 * GL], f32, tag="i", name=f"i{sg}")
                ot = gates.tile([128, 2 * GL], f32, tag="o", name=f"o{sg}")
                tht = gates.tile([128, 2 * GL], f32, tag="th", name=f"th{sg}")
                for gname, dst, fn, gm in (("u", ut, AF.Tanh, 0),
                                           ("i", it, AF.Sigmoid, 1),
                                           ("o", ot, AF.Sigmoid, 3)):
                    for half in range(2):
                        nc.scalar.activation(
                            dst[:, half * GL:(half + 1) * GL],
                            xg[gname][:, half * GL:(half + 1) * GL],
                            fn, bias=bx_fm[:, gm * 2 + half:gm * 2 + half + 1])
                cs = c0_3[:, :, sg * GL:(sg + 1) * GL]
                u3, i3, o3, th3 = v2(ut), v2(it), v2(ot), v2(tht)
                nc.vector.tensor_mul(cs, i3, u3)
                nc.scalar.activation(th3, cs, AF.Tanh)
                nc.vector.tensor_mul(hev3[:, :, sg * 256:(sg + 1) * 256],
                                     o3[:, :, 0::2], th3[:, :, 0::2])
                nc.vector.tensor_mul(hod3[:, :, sg * 256:(sg + 1) * 256],
                                     o3[:, :, 1::2], th3[:, :, 1::2])

            # ---- px = pad_row @ Wx.T + bx ----
            px_ps = psum.tile([1, 1024], f32, tag="g")
            for nh in range(2):
                for k in range(3):
                    r = 128 if k < 2 else IN_DIM - 256
                    nc.tensor.matmul(
                        px_ps[:, nh * 512:(nh + 1) * 512],
                        padT_sb[0:r, k:k + 1],
                        WxT_sb[0:r, k * 1024 + nh * 512:k * 1024 + (nh + 1) * 512],
                        start=(k == 0), stop=(k == 2))
            px_sb = const.tile([1, 1024], f32)
            nc.vector.tensor_add(px_sb[:, :], px_ps[:, :], bx_sb[:, :])
            px_fm = const.tile([128, 8], f32)
            for m in range(8):
                tp = psum.tile([128, 1], f32, tag="tp", name=f"pxt{m}")
                nc.tensor.transpose(tp[:, :], px_sb[0:1, m * 128:(m + 1) * 128],
                                    eye_sb[0:1, 0:1])
                nc.scalar.copy(px_fm[:, m:m + 1], tp[:, :])
            px5 = const.tile([1, 1280], f32r)  # node-major 5-gate pad row
            for g in range(5):
                nc.vector.tensor_copy(
                    px5[0:1, 256 * g:256 * (g + 1)],
                    px_sb[0:1, _PX5SRC[g]:_PX5SRC[g] + 256])

            # ---- feature-major level (B >= 256) ----
            def fm_level(cp, hev_p, hod_p, Bp, lvl, split_c):
                B = Bp // 2
                hev_n = state.tile([128, 2 * (B // 2)], f32r, tag="hev",
                                   name=f"hev{lvl}")
                hod_n = state.tile([128, 2 * (B // 2)], f32r, tag="hod",
                                   name=f"hod{lvl}")
                if split_c:
                    cev = state.tile([128, 2 * (B // 2)], f32, tag="cev",
                                     name=f"cev{lvl}", bufs=1)
                    cod = state.tile([128, 2 * (B // 2)], f32, tag="cod",
                                     name=f"cod{lvl}", bufs=1)
                else:
                    cn = state.tile([128, 2 * B], f32, tag="c", name=f"c{lvl}")
                cp3 = v2(cp)
                for g0 in range(0, B, 256):
                    G = min(256, B - g0)
                    gt = []
                    for gi in range(5):
                        t = psum.tile([128, 2 * G], f32, tag="g",
                                      name=f"g{lvl}_{g0}_{gi}")
                        for half in range(2):
                            m = gi * 2 + half
                            dst = t[:, half * G:(half + 1) * G]
                            for ki in range(4):
                                W = WlT_sb if ki < 2 else WrT_sb
                                kc = ki % 2
                                hp = hev_p if ki < 2 else hod_p
                                nc.tensor.matmul(
                                    dst,
                                    W[:, kc * 1280 + m * 128:
                                      kc * 1280 + (m + 1) * 128],
                                    v2(hp)[:, kc, g0:g0 + G],
                                    start=(ki == 0), stop=(ki == 3))
                        gt.append(t)
                    sfx = f"{lvl}_{g0}"
                    ut = gates.tile([128, 2 * G], f32, tag="u", name=f"u{sfx}")
                    it = gates.tile([128, 2 * G], f32, tag="i", name=f"i{sfx}")
                    lft = gates.tile([128, 2 * G], f32, tag="lf", name=f"lf{sfx}")
                    rft = gates.tile([128, 2 * G], f32, tag="rf", name=f"rf{sfx}")
                    ot = gates.tile([128, 2 * G], f32, tag="o", name=f"o{sfx}")
                    tht = gates.tile([128, 2 * G], f32, tag="th", name=f"th{sfx}")
                    x1 = gates.tile([128, 2 * G], f32, tag="x1", name=f"x1{sfx}", bufs=1)
                    x2 = gates.tile([128, 2 * G], f32, tag="x2", name=f"x2{sfx}", bufs=1)
                    x3 = gates.tile([128, 2 * G], f32, tag="x3", name=f"x3{sfx}", bufs=1)
                    s1 = gates.tile([128, 2 * G], f32, tag="s1", name=f"s1{sfx}", bufs=1)
                    for gi, (dst, fn) in enumerate((
                            (ut, AF.Tanh), (it, AF.Sigmoid), (lft, AF.Sigmoid),
                            (rft, AF.Sigmoid), (ot, AF.Sigmoid))):
                        for half in range(2):
                            m = gi * 2 + half
                            nc.scalar.activation(
                                dst[:, half * G:(half + 1) * G],
                                gt[gi][:, half * G:(half + 1) * G],
                                fn, bias=px_fm[:, _PXCOL[m]:_PXCOL[m] + 1])
                    lc = cp3[:, :, 2 * g0:2 * (g0 + G):2]
                    rc = cp3[:, :, 2 * g0 + 1:2 * (g0 + G):2]
                    u3, i3 = v2(ut), v2(it)
                    lf3, rf3, o3, th3 = v2(lft), v2(rft), v2(ot), v2(tht)
                    x13, x23, x33, s13 = v2(x1), v2(x2), v2(x3), v2(s1)
                    nc.vector.tensor_mul(x13, i3, u3)
                    nc.vector.tensor_mul(x23, lf3, lc)
                    nc.vector.tensor_mul(x33, rf3, rc)
                    nc.vector.tensor_add(s13, x13, x23)
                    if split_c:
                        ce = v2(cev)[:, :, g0 // 2:(g0 + G) // 2]
                        co = v2(cod)[:, :, g0 // 2:(g0 + G) // 2]
                        nc.vector.tensor_add(ce, s13[:, :, 0::2], x33[:, :, 0::2])
                        nc.vector.tensor_add(co, s13[:, :, 1::2], x33[:, :, 1::2])
                        nc.scalar.activation(th3[:, :, 0::2], ce, AF.Tanh)
                        nc.scalar.activation(th3[:, :, 1::2], co, AF.Tanh)
                    else:
                        cs = v2(cn)[:, :, g0:g0 + G]
                        nc.vector.tensor_add(cs, s13, x33)
                        nc.scalar.activation(th3, cs, AF.Tanh)
                    nc.vector.tensor_mul(v2(hev_n)[:, :, g0 // 2:(g0 + G) // 2],
                                         o3[:, :, 0::2], th3[:, :, 0::2])
                    nc.vector.tensor_mul(v2(hod_n)[:, :, g0 // 2:(g0 + G) // 2],
                                         o3[:, :, 1::2], th3[:, :, 1::2])
                if split_c:
                    return (cev, cod), hev_n, hod_n, B
                return cn, hev_n, hod_n, B

            # ---- node-major level (B <= 128) ----
            # lcrc: [B, 512] tile, cols [0:256]=lc, [256:512]=rc
            def nm_level(lcrc, hev_p, hod_p, B, lvl, last, ntot=None, hoff=0,
                         tg=""):
                if ntot is None:
                    ntot = B
                g_ps = psum.tile([128, 1280], f32, tag="g", name=f"gn{lvl}{tg}")
                for n0, nw in ((0, 512), (512, 512), (1024, 256)):
                    for ki in range(5):
                        if ki < 4:
                            par, kc = ki // 2, ki % 2
                            hsrc = hev_p if par == 0 else hod_p
                            lhsT = hsrc[:, kc * ntot + hoff:kc * ntot + hoff + B]
                            W = WlT_sb if par == 0 else WrT_sb
                            rhs = W[:, kc * 1280 + n0:kc * 1280 + n0 + nw]
                        else:
                            lhsT = ones_sb[0:1, 0:B]
                            rhs = px5[0:1, n0:n0 + nw]
                        nc.tensor.matmul(g_ps[0:B, n0:n0 + nw], lhsT, rhs,
                                         start=(ki == 0), stop=(ki == 4))
                sfx = f"n{lvl}{tg}"
                ut = gates.tile([128, 256], f32, tag=f"u{tg}", name=f"u{sfx}", bufs=1)
                sig = gates.tile([128, 1024], f32, tag=f"sg{tg}", name=f"sg{sfx}", bufs=1)
                tht = gates.tile([128, 256], f32, tag=f"th{tg}", name=f"th{sfx}", bufs=1)
                x1 = gates.tile([128, 256], f32, tag=f"x1{tg}", name=f"x1{sfx}", bufs=1)
                x23 = gates.tile([128, 512], f32, tag=f"x23{tg}", name=f"x23{sfx}", bufs=1)
                s1 = gates.tile([128, 256], f32, tag=f"s1{tg}", name=f"s1{sfx}", bufs=1)
                c_nm = state.tile([128, 256], f32, tag=f"cn{tg}", name=f"cn{sfx}")
                h_nm = state.tile([128, 256], f32, tag=f"hn{tg}", name=f"hn{sfx}")
                nc.scalar.activation(ut[0:B, :], g_ps[0:B, 0:256], AF.Tanh)
                nc.scalar.activation(sig[0:B, 0:256], g_ps[0:B, 256:512],
                                     AF.Sigmoid)
                nc.scalar.activation(sig[0:B, 256:768], g_ps[0:B, 512:1024],
                                     AF.Sigmoid)
                nc.scalar.activation(sig[0:B, 768:1024], g_ps[0:B, 1024:1280],
                                     AF.Sigmoid)
                nc.vector.tensor_mul(x1[0:B, :], sig[0:B, 0:256], ut[0:B, :])
                nc.vector.tensor_mul(x23[0:B, :], sig[0:B, 256:768], lcrc[0:B, :])
                nc.vector.tensor_add(s1[0:B, :], x1[0:B, :], x23[0:B, 0:256])
                nc.vector.tensor_add(c_nm[0:B, :], s1[0:B, :], x23[0:B, 256:512])
                nc.scalar.activation(tht[0:B, :], c_nm[0:B, :], AF.Tanh)
                nc.vector.tensor_mul(h_nm[0:B, :], sig[0:B, 768:1024], tht[0:B, :])
                if last:
                    return c_nm, h_nm, None, None
                hev_n = state.tile([128, 2 * (B // 2)], f32r, tag=f"hev{tg}",
                                   name=f"hev{lvl}{tg}")
                hod_n = state.tile([128, 2 * (B // 2)], f32r, tag=f"hod{tg}",
                                   name=f"hod{lvl}{tg}")
                for kc in range(2):
                    tp = psum.tile([128, B], f32, tag="tp", name=f"tph{lvl}{tg}_{kc}")
                    nc.tensor.transpose(tp[:, :],
                                        h_nm[0:B, 128 * kc:128 * (kc + 1)],
                                        eye_sb[0:B, 0:B])
                    nc.vector.tensor_copy(
                        hev_n[:, kc * (B // 2):(kc + 1) * (B // 2)],
                        tp[:, 0:B:2])
                    nc.vector.tensor_copy(
                        hod_n[:, kc * (B // 2):(kc + 1) * (B // 2)],
                        tp[:, 1:B:2])
                return c_nm, h_nm, hev_n, hod_n

            def gather_children(c_src, B, lvl, tg=""):
                lcrc = gates.tile([128, 512], f32, tag=f"lcrc{tg}",
                                  name=f"lcrc{lvl}{tg}")
                nc.sync.dma_start(lcrc[0:B, 0:256], c_src[0:2 * B:2, :])
                nc.sync.dma_start(lcrc[0:B, 256:512], c_src[1:2 * B:2, :])
                return lcrc

            # lvl0 (1024->512, FM, contiguous c), lvl1 (512->256, FM, split c)
            c_lvl0, hev, hod, B = fm_level(c0, hev, hod, LPC, 0, False)
            (cev1, cod1), hev, hod, B = fm_level(c_lvl0, hev, hod, B, 1, True)

            # boundary: transpose split FM c into node-major lcrc for lvl2
            lcrc = gates.tile([128, 512], f32, tag="lcrcA", name="lcrc2")
            for par, src in ((0, cev1), (1, cod1)):
                for kc in range(2):
                    tp = psum.tile([128, 128], f32, tag="tp",
                                   name=f"tpb{par}_{kc}")
                    nc.tensor.transpose(tp[:, :], v2(src)[:, kc, :],
                                        eye_sb[:, :])
                    nc.vector.tensor_copy(
                        lcrc[:, 256 * par + 128 * kc:256 * par + 128 * (kc + 1)],
                        tp[:, :])

            # lvl2..lvl9 node-major (B = 128..1)
            hevp, hodp, ntot = hev, hod, 128
            for lvl in range(2, 10):
                B >>= 1  # 128, 64, ..., 1
                last = (lvl == 9)
                c_nm, h_nm, hev_n, hod_n = nm_level(lcrc, hevp, hodp, B, lvl,
                                                    last, ntot=ntot, tg="A")
                if not last:
                    hevp, hodp, ntot = hev_n, hod_n, B // 2
                    lcrc = gather_children(c_nm, B // 2, lvl + 1, "A")

            # ---- write this core's subtree root (c, h) ----
            nc.sync.dma_start(out[0:1, :], c_nm[0:1, :])
            nc.sync.dma_start(out[1:2, :], h_nm[0:1, :])

    nc.compile()
    return nc


def _get_nc():
    if "nc" not in _CACHE:
        _CACHE["nc"] = _build()
    return _CACHE["nc"]


def kernel(embs, Wx, bx, Wl, Wr, emb_table, _trace=False, _trace_kwargs=None):
    from concourse.bass_utils import run_bass_kernel_spmd

    embs = np.ascontiguousarray(np.asarray(embs, dtype=np.float32))
    Wx = np.asarray(Wx, dtype=np.float32)
    bx = np.asarray(bx, dtype=np.float32)
    Wl = np.asarray(Wl, dtype=np.float32)
    Wr = np.asarray(Wr, dtype=np.float32)
    emb_table = np.asarray(emb_table, dtype=np.float32)

    WxT = np.ascontiguousarray(Wx.T)
    WlT = np.ascontiguousarray(Wl.T)
    WrT = np.ascontiguousarray(Wr.T)
    bxr = np.ascontiguousarray(bx.reshape(1, 1024))
    padT = np.ascontiguousarray(emb_table[-1].reshape(IN_DIM, 1))
    eye = np.eye(128, dtype=np.float32)
    ones = np.ones((1, 128), dtype=np.float32)

    in_maps = []
    for d in range(N_CORES):
        shard = np.ascontiguousarray(embs[d * LPC:(d + 1) * LPC].T)
        in_maps.append({
            "embsT": shard, "WxT": WxT, "WlT": WlT, "WrT": WrT,
            "bxr": bxr, "padT": padT, "eye_in": eye, "ones_in": ones,
        })

    nc = _get_nc()
    res = run_bass_kernel_spmd(nc, in_maps, list(range(N_CORES)),
                               trace=_trace, **(_trace_kwargs or {}))
    _CACHE["last_result"] = res

    # unshard: combine the 8 subtree roots (3 merge levels, 7 nodes)
    roots = [np.asarray(res.results[d]["out"], dtype=np.float32)
             for d in range(N_CORES)]
    c = np.stack([r[0] for r in roots])  # [8, 256]
    h = np.stack([r[1] for r in roots])
    px = emb_table[-1] @ WxT + bx        # [1024]
    m = MEM_DIM

    def sig(x):
        return 1.0 / (1.0 + np.exp(-x))

    while c.shape[0] > 1:
        lg = h[0::2] @ WlT
        rg = h[1::2] @ WrT
        u = np.tanh(px[0:m] + lg[:, 0:m] + rg[:, 0:m])
        i = sig(px[m:2 * m] + lg[:, m:2 * m] + rg[:, m:2 * m])
        lf = sig(px[2 * m:3 * m] + lg[:, 2 * m:3 * m] + rg[:, 2 * m:3 * m])
        rf = sig(px[2 * m:3 * m] + lg[:, 3 * m:4 * m] + rg[:, 3 * m:4 * m])
        o = sig(px[3 * m:4 * m] + lg[:, 4 * m:5 * m] + rg[:, 4 * m:5 * m])
        c = i * u + lf * c[0::2] + rf * c[1::2]
        h = o * np.tanh(c)
    return np.stack([c, h]).astype(np.float32)



# revision 6
# speedup vs baseline: 2.2113x; 2.2113x over previous
"""BinaryTreeLSTM on 8 Trainium2 NeuronCores — feature-major fp16 pipeline.

Data-parallel over the leaf batch: core d owns leaves [1024d, 1024d+1024)
as 8 independent 128-leaf subtrees, folded through the leaf LSTM plus 7
merge levels (B = 512, 256, 128, 64, 32, 16, 8). The 64 subtree roots
(8 per core) are gathered on host, which folds the remaining 6 levels
(63 nodes, <1% of FLOPs).

Everything stays feature-major ([128 feat-partitions, 2 halves, B nodes])
on every level:
- matmul: stationary = weight chunk [128, 128 gate cols] (fp16 -> FWL),
  moving = child h tile [128, B]. fp16 operands avoid the fp32r 4x
  penalty below 256 moving columns, so the small tail levels stay cheap.
- even/odd child splits are free-dim stride-2 accesses (no transposes,
  no SBUF-SBUF DMA between levels).
- c stays fp32 (root values reach ~1.5e3); gates stay fp32, h is fp16.
- K=300 operands are host-padded to 384 rows so every K-chunk is a full
  128 partitions (zero rows contribute exactly 0).
- px (pad-embedding x-projection) and bias columns are host-precomputed.
"""

import numpy as np

IN_DIM = 300
KPAD = 384
MEM_DIM = 256
N_LEAVES = 8192
N_CORES = 8
LPC = N_LEAVES // N_CORES  # 1024 leaves per core
N_SUB = 8                  # subtrees per core -> 8 roots per core

# (gate, half) -> px m-chunk column ([u,i,lf,rf,o] x 2; lf/rf share fx)
_PXCOL = [0, 1, 2, 3, 4, 5, 4, 5, 6, 7]

_CACHE = {}


def _build():
    import concourse.bacc as bacc
    import concourse.mybir as mybir
    import concourse.tile as tile

    f32 = mybir.dt.float32
    fp16 = mybir.dt.float16
    AF = mybir.ActivationFunctionType

    nc = bacc.Bacc("TRN2", target_bir_lowering=False, debug=False,
                   num_devices=N_CORES)

    embsT = nc.dram_tensor("embsT", [KPAD, LPC], fp16, kind="ExternalInput").ap()
    WxT = nc.dram_tensor("WxT", [KPAD, 768], fp16, kind="ExternalInput").ap()
    WlT = nc.dram_tensor("WlT", [MEM_DIM, 1280], fp16, kind="ExternalInput").ap()
    WrT = nc.dram_tensor("WrT", [MEM_DIM, 1280], fp16, kind="ExternalInput").ap()
    bxf = nc.dram_tensor("bxf", [128, 6], f32, kind="ExternalInput").ap()
    pxf = nc.dram_tensor("pxf", [128, 10], f32, kind="ExternalInput").ap()
    out = nc.dram_tensor("out", [4, 128 * N_SUB], f32,
                         kind="ExternalOutput").ap()

    with tile.TileContext(nc) as tc:
        with (
            tc.tile_pool(name="const", bufs=1) as const,
            tc.tile_pool(name="state", bufs=1) as state,
            tc.tile_pool(name="gates", bufs=2) as gates,
            tc.tile_pool(name="psum", bufs=1, space="PSUM") as psum,
        ):
            v2 = lambda t: t.rearrange("p (c n) -> p c n", c=2)

            # ---- PE warm-up on a zeroed scratch (no DMA dependency) ----
            warm = const.tile([128, 512], fp16)
            nc.vector.memset(warm[:, :], 0.0)
            wps = psum.tile([128, 512], f32, tag="warm")
            for wi in range(9):
                nc.tensor.matmul(wps[:, :], warm[:, 0:128], warm[:, :],
                                 start=(wi == 0), stop=(wi == 8))

            # ---- constants; 3 HWDGE queues in parallel ----
            Wx_sb = const.tile([128, 3 * 768], fp16)
            nc.sync.dma_start(
                Wx_sb.rearrange("p (k f) -> p k f", k=3),
                WxT.rearrange("(k p) f -> p k f", p=128))
            bx_fm = const.tile([128, 6], f32)
            nc.sync.dma_start(bx_fm[:, :], bxf[:, :])
            px_fm = const.tile([128, 10], f32)
            nc.sync.dma_start(px_fm[:, :], pxf[:, :])

            embs_sb = const.tile([128, 3 * LPC], fp16)
            e3 = embs_sb.rearrange("p (k n) -> p k n", k=3)
            ed = embsT.rearrange("(k p) n -> p k n", p=128)
            for g in range(4):
                nc.scalar.dma_start(e3[:, :, g * 256:(g + 1) * 256],
                                    ed[:, :, g * 256:(g + 1) * 256])

            Wl_sb = const.tile([128, 2 * 1280], fp16)
            Wr_sb = const.tile([128, 2 * 1280], fp16)
            nc.sync.dma_start(
                Wl_sb.rearrange("p (k f) -> p k f", k=2),
                WlT.rearrange("(k p) f -> p k f", p=128))
            nc.sync.dma_start(
                Wr_sb.rearrange("p (k f) -> p k f", k=2),
                WrT.rearrange("(k p) f -> p k f", p=128))

            # ---- leaf phase: B=1024, 4 chunks of 256 ----
            c0 = state.tile([128, 2 * LPC], f32, tag="c0")
            hev = state.tile([128, 2 * 512], fp16, tag="h0e")
            hod = state.tile([128, 2 * 512], fp16, tag="h0o")
            c0_3, hev3, hod3 = v2(c0), v2(hev), v2(hod)
            for sg in range(4):
                gt = {}
                for gi, gname in enumerate(("u", "i", "o")):
                    t = psum.tile([128, 2 * 256], f32, tag=f"mg{gi}",
                                  name=f"x{gname}{sg}")
                    for half in range(2):
                        m = gi * 2 + half
                        dst = t[:, half * 256:(half + 1) * 256]
                        for ki in range(3):
                            nc.tensor.matmul(
                                dst,
                                Wx_sb[:, ki * 768 + m * 128:
                                      ki * 768 + (m + 1) * 128],
                                embs_sb[:, ki * LPC + sg * 256:
                                        ki * LPC + (sg + 1) * 256],
                                start=(ki == 0), stop=(ki == 2))
                    gt[gname] = t
                ut = gates.tile([128, 2 * 256], f32, tag="mu", name=f"u{sg}")
                it = gates.tile([128, 2 * 256], f32, tag="mi", name=f"i{sg}")
                ot = gates.tile([128, 2 * 256], f32, tag="mo", name=f"o{sg}")
                tht = gates.tile([128, 2 * 256], f32, tag="mth", name=f"th{sg}")
                for gi, (gname, dst, fn) in enumerate((
                        ("u", ut, AF.Tanh), ("i", it, AF.Sigmoid),
                        ("o", ot, AF.Sigmoid))):
                    for half in range(2):
                        bc = gi * 2 + half
                        nc.scalar.activation(
                            dst[:, half * 256:(half + 1) * 256],
                            gt[gname][:, half * 256:(half + 1) * 256],
                            fn, bias=bx_fm[:, bc:bc + 1])
                cs = c0_3[:, :, sg * 256:(sg + 1) * 256]
                u3, i3, o3, th3 = v2(ut), v2(it), v2(ot), v2(tht)
                nc.vector.tensor_mul(cs, i3, u3)
                nc.scalar.activation(th3, cs, AF.Tanh)
                nc.vector.tensor_mul(hev3[:, :, sg * 128:(sg + 1) * 128],
                                     o3[:, :, 0::2], th3[:, :, 0::2])
                nc.vector.tensor_mul(hod3[:, :, sg * 128:(sg + 1) * 128],
                                     o3[:, :, 1::2], th3[:, :, 1::2])

            # ---- merge levels, all feature-major ----
            GATE_FNS = (AF.Tanh, AF.Sigmoid, AF.Sigmoid, AF.Sigmoid,
                        AF.Sigmoid)

            def fm_level(cp, hev_p, hod_p, B, lvl):
                """children: cp [128,2,2B] f32, hev_p/hod_p [128,2,B] fp16.
                Returns (c, hev, hod) of this level's B nodes; at the last
                level (B == N_SUB) returns (c, h, None) with h unsplit."""
                last = B == N_SUB
                cn = state.tile([128, 2 * B], f32, tag=f"c{lvl}")
                if last:
                    hn = state.tile([128, 2 * B], f32, tag=f"h{lvl}")
                else:
                    hev_n = state.tile([128, B], fp16, tag=f"h{lvl}e")
                    hod_n = state.tile([128, B], fp16, tag=f"h{lvl}o")
                for g0 in range(0, B, 256):
                    G = min(256, B - g0)
                    sfx = f"{lvl}_{g0}"
                    gt = []
                    for gi in range(5):
                        t = psum.tile([128, 2 * G], f32, tag=f"mg{gi}",
                                      name=f"g{sfx}_{gi}")
                        for half in range(2):
                            m = gi * 2 + half
                            dst = t[:, half * G:(half + 1) * G]
                            for ki in range(4):
                                W = Wl_sb if ki < 2 else Wr_sb
                                kc = ki % 2
                                hp = hev_p if ki < 2 else hod_p
                                nc.tensor.matmul(
                                    dst,
                                    W[:, kc * 1280 + m * 128:
                                      kc * 1280 + (m + 1) * 128],
                                    hp[:, kc, g0:g0 + G],
                                    start=(ki == 0), stop=(ki == 3))
                        gt.append(t)
                    ga = []
                    for gi, gname in enumerate(("u", "i", "lf", "rf", "o")):
                        a = gates.tile([128, 2 * G], f32, tag=f"m{gname}",
                                       name=f"{gname}{sfx}")
                        for half in range(2):
                            pc = gi * 2 + half  # pxf is pre-permuted
                            nc.scalar.activation(
                                a[:, half * G:(half + 1) * G],
                                gt[gi][:, half * G:(half + 1) * G],
                                GATE_FNS[gi], bias=px_fm[:, pc:pc + 1])
                        ga.append(v2(a))
                    u3, i3, lf3, rf3, o3 = ga
                    lc = cp[:, :, 2 * g0:2 * (g0 + G):2]
                    rc = cp[:, :, 2 * g0 + 1:2 * (g0 + G):2]
                    x1 = gates.tile([128, 2 * G], f32, tag="x1", name=f"x1{sfx}")
                    x2 = gates.tile([128, 2 * G], f32, tag="x2", name=f"x2{sfx}")
                    x3 = gates.tile([128, 2 * G], f32, tag="x3", name=f"x3{sfx}")
                    s1 = gates.tile([128, 2 * G], f32, tag="s1", name=f"s1{sfx}")
                    tht = gates.tile([128, 2 * G], f32, tag="mth",
                                     name=f"th{sfx}")
                    x13, x23, x33, s13, th3 = v2(x1), v2(x2), v2(x3), v2(s1), v2(tht)
                    cs = v2(cn)[:, :, g0:g0 + G]
                    nc.vector.tensor_mul(x13, i3, u3)
                    nc.vector.tensor_mul(x23, lf3, lc)
                    nc.vector.tensor_mul(x33, rf3, rc)
                    nc.vector.tensor_add(s13, x13, x23)
                    nc.vector.tensor_add(cs, s13, x33)
                    nc.scalar.activation(th3, cs, AF.Tanh)
                    if last:
                        nc.vector.tensor_mul(v2(hn)[:, :, g0:g0 + G], o3, th3)
                    else:
                        nc.vector.tensor_mul(
                            v2(hev_n)[:, :, g0 // 2:(g0 + G) // 2],
                            o3[:, :, 0::2], th3[:, :, 0::2])
                        nc.vector.tensor_mul(
                            v2(hod_n)[:, :, g0 // 2:(g0 + G) // 2],
                            o3[:, :, 1::2], th3[:, :, 1::2])
                if last:
                    return cn, hn, None
                return cn, hev_n, hod_n

            cp, he, ho = c0, hev, hod
            B, lvl = 512, 1
            while B >= N_SUB:
                cp, he, ho = fm_level(v2(cp), v2(he), v2(ho) if ho is not None
                                      else None, B, lvl)
                B >>= 1
                lvl += 1

            # cp = root c [128, 2*N_SUB] f32, he = root h (unsplit) f32
            for half in range(2):
                nc.sync.dma_start(out[0 + half:1 + half, :],
                                  v2(cp)[:, half, :])
                nc.sync.dma_start(out[2 + half:3 + half, :],
                                  v2(he)[:, half, :])

    nc.compile()
    return nc


def _get_nc():
    if "nc" not in _CACHE:
        _CACHE["nc"] = _build()
    return _CACHE["nc"]


def kernel(embs, Wx, bx, Wl, Wr, emb_table, _trace=False, _trace_kwargs=None):
    from concourse.bass_utils import run_bass_kernel_spmd

    fp16 = np.float16
    embs = np.asarray(embs, dtype=np.float32)
    Wx = np.asarray(Wx, dtype=np.float32)
    bx = np.asarray(bx, dtype=np.float32)
    Wl = np.asarray(Wl, dtype=np.float32)
    Wr = np.asarray(Wr, dtype=np.float32)
    emb_table = np.asarray(emb_table, dtype=np.float32)

    # u(cx), i(ix), o(ox) gate rows of Wx, transposed + K-padded to 384
    Wxuio = np.concatenate([Wx[0:256], Wx[256:512], Wx[768:1024]], axis=0)
    WxT = np.zeros((KPAD, 768), dtype=fp16)
    WxT[:IN_DIM] = Wxuio.T.astype(fp16)
    WlT = np.ascontiguousarray(Wl.T.astype(fp16))
    WrT = np.ascontiguousarray(Wr.T.astype(fp16))

    # bias columns: leaf bx (u,i,o m-chunks) and merge px (5 gates x 2)
    bxm = bx.reshape(8, 128)
    bxf = np.ascontiguousarray(bxm[[0, 1, 2, 3, 6, 7]].T)  # [128, 6]
    px = emb_table[-1] @ Wx.T + bx                         # [1024]
    pxm = px.reshape(8, 128)
    pxf = np.ascontiguousarray(pxm[_PXCOL].T)              # [128, 10]

    in_maps = []
    for d in range(N_CORES):
        shard = np.zeros((KPAD, LPC), dtype=fp16)
        shard[:IN_DIM] = embs[d * LPC:(d + 1) * LPC].T.astype(fp16)
        in_maps.append({
            "embsT": shard, "WxT": WxT, "WlT": WlT, "WrT": WrT,
            "bxf": bxf, "pxf": pxf,
        })

    nc = _get_nc()
    res = run_bass_kernel_spmd(nc, in_maps, list(range(N_CORES)),
                               trace=_trace, **(_trace_kwargs or {}))
    _CACHE["last_result"] = res

    # unshard: 64 subtree roots -> 6 numpy merge levels (63 nodes)
    cs, hs = [], []
    for d in range(N_CORES):
        o = np.asarray(res.results[d]["out"], dtype=np.float32)
        # row (s*2 + half) holds [128, N_SUB]: feat = half*128 + p
        cs.append(o[0:2].reshape(256, N_SUB).T)
        hs.append(o[2:4].reshape(256, N_SUB).T)
    c = np.concatenate(cs, axis=0)  # [64, 256]
    h = np.concatenate(hs, axis=0)
    WlTf = Wl.T.astype(np.float32)
    WrTf = Wr.T.astype(np.float32)
    m = MEM_DIM

    def sig(x):
        return 1.0 / (1.0 + np.exp(-x))

    while c.shape[0] > 1:
        lg = h[0::2] @ WlTf
        rg = h[1::2] @ WrTf
        u = np.tanh(px[0:m] + lg[:, 0:m] + rg[:, 0:m])
        i = sig(px[m:2 * m] + lg[:, m:2 * m] + rg[:, m:2 * m])
        lf = sig(px[2 * m:3 * m] + lg[:, 2 * m:3 * m] + rg[:, 2 * m:3 * m])
        rf = sig(px[2 * m:3 * m] + lg[:, 3 * m:4 * m] + rg[:, 3 * m:4 * m])
        o = sig(px[3 * m:4 * m] + lg[:, 4 * m:5 * m] + rg[:, 4 * m:5 * m])
        c = i * u + lf * c[0::2] + rf * c[1::2]
        h = o * np.tanh(c)
    return np.stack([c, h]).astype(np.float32)
